# revision 10
# baseline (speedup 1.0000x reference)
"""ConvCapsuleLayer Trainium2 kernel (v2).

Strategy:
  - Data-parallel over batch B=16 across 8 cores (B_local=2 per core).
  - Conv (5x5, SAME, Ai=32 -> Co*Ao=256) done on the PE as x-stationary
    matmuls: lhsT = shifted x patches [K=(tap,ai), M=128 pixels],
    rhs = repacked W [K, 256], accumulated over 7 tap-groups in PSUM.
    Conv emitted in (bb, half) chunks so each routing quarter's votes
    complete at ~25/50/75/100% of the conv, keeping DVE fed early.
  - Votes layout [pix, g, ao, ci, co] fp16 (co innermost): every big DVE
    multiply hits 2x mode (stride-1 innermost on all operands) and all
    reductions (over ci for preact, over ao for agreement/norm) are
    contiguous tree-adds instead of strided tensor_reduces.
  - Dynamic routing (3 iters) per pixel on-chip: softmax over co (f32
    logits/exp for range safety), preact = sum_ci route*votes + bias,
    squash factor per (pixel, co), agreement = sum_ao votes*preact with
    the squash factor folded in afterwards (no act materialization in
    iters 1-2). sqrt via exp(0.5*ln(x)).
  - Iteration-1 shortcut: route is uniform 1/16, so preact1 rides the conv
    via an extra "sum over ci" input plane (computed host-side).
  - PSUM->SBUF vote copies alternate between ACT and Pool engines.
  - Output fp16 in (ao, co) order; host transposes to (co, ao) and casts.
"""

import os
import sys

import numpy as np

sys.path.insert(0, "/opt/trn_rl_repo")

from contextlib import ExitStack

import concourse.bacc as bacc
import concourse.bass as bass
import concourse.mybir as mybir
import concourse.tile as tile
from concourse.bass_utils import run_bass_kernel_spmd

F16 = mybir.dt.float16
BF16 = mybir.dt.bfloat16
F32 = mybir.dt.float32
AX = mybir.AxisListType
OP = mybir.AluOpType
AF = mybir.ActivationFunctionType

N_CORES = 8
B_FULL, H, Wd, Ci, Ai = 16, 32, 32, 8, 32
K, Co, Ao = 5, 16, 16
B_LOC = B_FULL // N_CORES  # 2
NQ = 4  # 4 routing quarters of 4 row-quad groups each

_cache = {}


def _build_program():
    nc = bacc.Bacc(None, target_bir_lowering=False)
    xpad_d = nc.dram_tensor(
        "xpad", [B_LOC, Ci, 2, 4, Ai, 36, 32], F16, kind="ExternalInput"
    )
    wstk_d = nc.dram_tensor("wstk", [7, 128, 256], F16, kind="ExternalInput")
    bias_d = nc.dram_tensor("biasin", [128, 2, 256], F32, kind="ExternalInput")
    out_d = nc.dram_tensor("out", [B_LOC, H, Wd, Ao, Co], F16, kind="ExternalOutput")

    with tile.TileContext(nc) as tc, ExitStack() as ctx:
        const_p = ctx.enter_context(tc.tile_pool(name="const", bufs=1))
        votes_p = ctx.enter_context(tc.tile_pool(name="votes", bufs=1))
        xrep_p = ctx.enter_context(tc.tile_pool(name="xrep", bufs=3))
        psum_p = ctx.enter_context(
            tc.tile_pool(name="psum", bufs=8, space=bass.MemorySpace.PSUM)
        )
        big_p = ctx.enter_context(tc.tile_pool(name="big", bufs=2))
        tree_p = ctx.enter_context(tc.tile_pool(name="tree", bufs=1))
        mid_p = ctx.enter_context(tc.tile_pool(name="mid", bufs=2))
        tiny_p = ctx.enter_context(tc.tile_pool(name="tiny", bufs=2))
        out_p = ctx.enter_context(tc.tile_pool(name="outs", bufs=2))

        # ---- constants ----
        wstk = const_p.tile([128, 7, 256], F16)
        for s in range(7):
            nc.sync.dma_start(wstk[:, s], wstk_d[s])
        bias_rep = const_p.tile([128, 2, 256], F32)
        nc.sync.dma_start(bias_rep[:], bias_d[:])
        # (ao, co) order; bias16 = 16*b (iter-1 raw preact), bias1 = b
        bias_h = const_p.tile([128, 2, 256], F16)
        nc.scalar.copy(bias_h[:], bias_rep[:])
        bias16 = bias_h[:, 0].rearrange("p (ao co) -> p ao co", ao=16)
        bias1 = bias_h[:, 1].rearrange("p (ao co) -> p ao co", ao=16)

        # votes per quarter: [pix, g, ao, ci, co] fp16 + ci-sum plane
        votes_q = [
            votes_p.tile([128, 4, 16, Ci, 16], F16, name=f"votes{q}")
            for q in range(NQ)
        ]

        ncopy = [0]

        def conv_quarter(bb, hf):
            """Conv for output rows 16*hf..16*hf+15 of batch bb -> quarter q."""
            q = 2 * bb + hf
            r0 = 16 * hf  # first padded row needed (rows r0..r0+19)
            for ci in range(Ci):
                xrep = xrep_p.tile([128, 20 * 32], F16, tag="xrepA")
                xrep2 = xrep_p.tile([128, 20 * 32], F16, tag="xrepB")
                src = xpad_d[bb, ci, 0].rearrange("s ai r c -> (s ai) (r c)")
                nc.sync.dma_start(xrep[:], src[:, r0 * 32 : (r0 + 20) * 32])
                src2 = xpad_d[bb, ci, 1].rearrange("s ai r c -> (s ai) (r c)")
                nc.sync.dma_start(xrep2[:], src2[:, r0 * 32 : (r0 + 20) * 32])
                for yq in range(4):
                    ps = psum_p.tile([128, 256], F32, tag="convps")
                    for dy in range(5):
                        o = (4 * yq + dy) * 32
                        nc.tensor.matmul(
                            ps[:],
                            xrep[:, o : o + 128],
                            wstk[:, dy],
                            start=(dy == 0),
                            stop=False,
                        )
                    o = 4 * yq * 32
                    nc.tensor.matmul(
                        ps[:], xrep2[:, o : o + 128], wstk[:, 5], start=False,
                        stop=False,
                    )
                    o = (4 * yq + 4) * 32
                    nc.tensor.matmul(
                        ps[:],
                        xrep2[0:32, o : o + 128],
                        wstk[0:32, 6],
                        start=False,
                        stop=True,
                    )
                    nc.scalar.copy(votes_q[q][:, yq, :, ci, :], ps[:])
                    ncopy[0] += 1

        def squash_factor(ns, scale):
            """fac[g, co] = sqrt(s2*ns)/(1+s2*ns), from ns fp16 [128,4,16]."""
            s2 = scale * scale
            lnv = tiny_p.tile([128, 4, 16], F32, tag="lnv")
            nc.scalar.activation(lnv[:], ns[:], AF.Ln, scale=s2)
            sqr = tiny_p.tile([128, 4, 16], F32, tag="sqr")
            nc.scalar.activation(sqr[:], lnv[:], AF.Exp, scale=0.5)
            onep = tiny_p.tile([128, 4, 16], F32, tag="onep")
            nc.vector.tensor_scalar(onep[:], ns[:], s2, 1.0, op0=OP.mult, op1=OP.add)
            rec = tiny_p.tile([128, 4, 16], F32, tag="rec")
            nc.vector.reciprocal(rec[:], onep[:])
            fac = tiny_p.tile([128, 4, 16], F32, tag="fac")
            nc.vector.tensor_tensor(fac[:], sqr[:], rec[:], OP.mult)
            return fac

        def ns_tree(sq):
            """ns[g, co] = sum_ao sq[g, ao, co], contiguous tree adds."""
            n1 = tiny_p.tile([128, 4, 8, 16], F16, tag="ns1")
            nc.vector.tensor_tensor(n1[:], sq[:, :, 0:8], sq[:, :, 8:16], OP.add)
            n2 = tiny_p.tile([128, 4, 4, 16], F16, tag="ns2")
            nc.vector.tensor_tensor(n2[:], n1[:, :, 0:4], n1[:, :, 4:8], OP.add)
            n3 = tiny_p.tile([128, 4, 2, 16], F16, tag="ns3")
            nc.vector.tensor_tensor(n3[:], n2[:, :, 0:2], n2[:, :, 2:4], OP.add)
            ns = tiny_p.tile([128, 4, 16], F16, tag="ns")
            nc.vector.tensor_tensor(ns[:], n3[:, :, 0], n3[:, :, 1], OP.add)
            return ns

        def agreement_tree(V, pb):
            """agr0[g, ci, co] f32 = sum_ao V * pb (pb broadcast over ci)."""
            t2 = big_p.tile([128, 4, 16, Ci, 16], F16, tag="big")
            pbb = pb[:].unsqueeze(3).broadcast_to([128, 4, 16, Ci, 16])
            nc.vector.tensor_tensor(t2[:], V, pbb, OP.mult)
            a1 = tree_p.tile([128, 4, 8, Ci, 16], F16, tag="atr1")
            nc.vector.tensor_tensor(a1[:], t2[:, :, 0:8], t2[:, :, 8:16], OP.add)
            a2 = tree_p.tile([128, 4, 4, Ci, 16], F16, tag="atr2")
            nc.vector.tensor_tensor(a2[:], a1[:, :, 0:4], a1[:, :, 4:8], OP.add)
            a3 = tree_p.tile([128, 4, 2, Ci, 16], F16, tag="atr3")
            nc.vector.tensor_tensor(a3[:], a2[:, :, 0:2], a2[:, :, 2:4], OP.add)
            agr0 = mid_p.tile([128, 4, Ci, 16], F32, tag="agr0")
            nc.vector.tensor_tensor(agr0[:], a3[:, :, 0], a3[:, :, 1], OP.add)
            return agr0

        def weighted_preact(V, r, bias_ap):
            """pb[g, ao, co] fp16 = sum_ci V * r (r broadcast over ao) + bias."""
            t1 = big_p.tile([128, 4, 16, Ci, 16], F16, tag="big")
            rb = r[:].unsqueeze(2).broadcast_to([128, 4, 16, Ci, 16])
            nc.vector.tensor_tensor(t1[:], V, rb, OP.mult)
            p1 = tree_p.tile([128, 4, 16, 4, 16], F16, tag="ptr1")
            nc.vector.tensor_tensor(
                p1[:], t1[:, :, :, 0:4], t1[:, :, :, 4:8], OP.add
            )
            p2 = tree_p.tile([128, 4, 16, 2, 16], F16, tag="ptr2")
            nc.vector.tensor_tensor(
                p2[:], p1[:, :, :, 0:2], p1[:, :, :, 2:4], OP.add
            )
            p3 = tree_p.tile([128, 4, 16, 16], F16, tag="ptr3")
            nc.vector.tensor_tensor(p3[:], p2[:, :, :, 0], p2[:, :, :, 1], OP.add)
            pb = mid_p.tile([128, 4, 16, 16], F16, tag="pb")
            bb_ = bias_ap.unsqueeze(1).broadcast_to([128, 4, 16, 16])
            nc.vector.tensor_tensor(pb[:], p3[:], bb_, OP.add)
            return pb

        def softmax_route(lg):
            """r[g, ci, co] fp16 = softmax over co of lg f32."""
            e = mid_p.tile([128, 4, Ci, 16], F32, tag="expv")
            nc.scalar.activation(e[:], lg[:], AF.Exp)
            den = tiny_p.tile([128, 4, Ci], F32, tag="den")
            nc.vector.tensor_reduce(den[:], e[:], axis=AX.X, op=OP.add)
            rc = tiny_p.tile([128, 4, Ci], F32, tag="rc")
            nc.vector.reciprocal(rc[:], den[:])
            r = mid_p.tile([128, 4, Ci, 16], F16, tag="route")
            rcb = rc[:].unsqueeze(3).broadcast_to([128, 4, Ci, 16])
            nc.vector.tensor_tensor(r[:], e[:], rcb, OP.mult)
            return r

        def routing_quarter(q):
            V = votes_q[q][:]
            inv16 = 1.0 / 16.0

            # ---- iter 1 (uniform route; raw preact = sum_ci V + 16*bias,
            # computed on-chip with the same ci tree as weighted_preact) ----
            s1 = tree_p.tile([128, 4, 16, 4, 16], F16, tag="ptr1")
            nc.vector.tensor_tensor(
                s1[:], V[:, :, :, 0:4], V[:, :, :, 4:8], OP.add
            )
            s2 = tree_p.tile([128, 4, 16, 2, 16], F16, tag="ptr2")
            nc.vector.tensor_tensor(
                s2[:], s1[:, :, :, 0:2], s1[:, :, :, 2:4], OP.add
            )
            s3 = tree_p.tile([128, 4, 16, 16], F16, tag="ptr3")
            nc.vector.tensor_tensor(s3[:], s2[:, :, :, 0], s2[:, :, :, 1], OP.add)
            pb1 = mid_p.tile([128, 4, 16, 16], F16, tag="pb")
            b16 = bias16.unsqueeze(1).broadcast_to([128, 4, 16, 16])
            nc.vector.tensor_tensor(pb1[:], s3[:], b16, OP.add)
            sq1 = mid_p.tile([128, 4, 16, 16], F16, tag="sq")
            nc.scalar.activation(sq1[:], pb1[:], AF.Square)
            ns1 = ns_tree(sq1)
            fac1 = squash_factor(ns1, inv16)
            agr1 = agreement_tree(V, pb1)
            # fold the iter-1 uniform-route scale into fac1 (tiny op)
            fac1s = tiny_p.tile([128, 4, 16], F32, tag="facs")
            nc.vector.tensor_scalar_mul(fac1s[:], fac1[:], inv16)
            logits1 = mid_p.tile([128, 4, Ci, 16], F32, tag="logits1")
            f1b = fac1s[:].unsqueeze(2).broadcast_to([128, 4, Ci, 16])
            nc.vector.tensor_tensor(logits1[:], agr1[:], f1b, OP.mult)

            # ---- iter 2 ----
            r2 = softmax_route(logits1)
            pb2 = weighted_preact(V, r2, bias1)
            sq2 = mid_p.tile([128, 4, 16, 16], F16, tag="sq")
            nc.scalar.activation(sq2[:], pb2[:], AF.Square)
            ns2 = ns_tree(sq2)
            fac2 = squash_factor(ns2, 1.0)
            agr2 = agreement_tree(V, pb2)
            upd = mid_p.tile([128, 4, Ci, 16], F32, tag="upd")
            f2b = fac2[:].unsqueeze(2).broadcast_to([128, 4, Ci, 16])
            nc.vector.tensor_tensor(upd[:], agr2[:], f2b, OP.mult)
            logits2 = mid_p.tile([128, 4, Ci, 16], F32, tag="logits2")
            nc.vector.tensor_tensor(logits2[:], logits1[:], upd[:], OP.add)

            # ---- iter 3 ----
            r3 = softmax_route(logits2)
            pb3 = weighted_preact(V, r3, bias1)
            sq3 = mid_p.tile([128, 4, 16, 16], F16, tag="sq")
            nc.scalar.activation(sq3[:], pb3[:], AF.Square)
            ns3 = ns_tree(sq3)
            fac3 = squash_factor(ns3, 1.0)
            act3 = out_p.tile([128, 4, 16, 16], F16, tag="actout")
            f3b = fac3[:].unsqueeze(2).broadcast_to([128, 4, 16, 16])
            nc.vector.tensor_tensor(act3[:], pb3[:], f3b, OP.mult)

            bb, half = divmod(q, 2)
            dst = out_d[bb, 16 * half : 16 * half + 16].rearrange(
                "(gg yy) x ao co -> (yy x) gg ao co", yy=4
            )
            # ACT queue, not sync: the sync queue carries the next quarter's
            # xrep loads and must not stall behind routing completion.
            nc.scalar.dma_start(dst, act3[:])

        # ---- pipeline: conv quarter then its routing ----
        for bb in range(B_LOC):
            for hf in range(2):
                conv_quarter(bb, hf)
                routing_quarter(2 * bb + hf)

    nc.compile()
    return nc


def _prep_core_inputs(x_core, W, b):
    f16 = np.float16
    xr = np.transpose(x_core, (0, 3, 4, 1, 2)).astype(f16)  # [B_LOC, Ci, Ai, H, W]
    planes = xr
    # xpad[b, 0, s, ai, ci, r, c] = plane[r-2, c+s-2]   (s = dx shift 0..3)
    # xpad[b, 1, g, ai, ci, r, c] = plane[r+g-2, c+2]   (g = dy shift 0..3, dx=4)
    xpad = np.zeros((B_LOC, Ci, 2, 4, Ai, 36, 32), dtype=f16)
    for s in range(4):
        c_lo = max(0, 2 - s)
        c_hi = min(32, 34 - s)
        xpad[:, :, 0, s, :, 2:34, c_lo:c_hi] = planes[
            :, :, :, :, c_lo + s - 2 : c_hi + s - 2
        ]
    for g in range(4):
        r_lo = max(0, 2 - g)
        r_hi = min(36, 34 - g)
        xpad[:, :, 1, g, :, r_lo:r_hi, 0:30] = planes[
            :, :, :, r_lo + g - 2 : r_hi + g - 2, 2:32
        ]
    # W stacks in (ao, co) output order:
    # slot dy (0..4): [(dx g, ai), 256]; slot 5: [(dy g, ai), 256] at dx=4;
    # slot 6: [ai, 256] for tap (4, 4).
    Wr = W.reshape(K, K, Ai, Co, Ao).transpose(0, 1, 2, 4, 3)  # [dy,dx,ai,ao,co]
    wstk = np.zeros((7, 128, 256), dtype=f16)
    for dy in range(5):
        wstk[dy] = (
            Wr[dy, 0:4].reshape(4 * Ai, Ao * Co).astype(f16)
        )  # [(dx,ai), (ao,co)]
    wstk[5] = Wr[0:4, 4].reshape(4 * Ai, Ao * Co).astype(f16)  # [(dy,ai), ...]
    wstk[6, :32] = Wr[4, 4].reshape(Ai, Ao * Co).astype(f16)
    bias_aoco = b[0, 0].T.reshape(256).astype(np.float32)  # (ao, co) order
    biasin = np.broadcast_to(
        np.stack([16.0 * bias_aoco, bias_aoco])[None], (128, 2, 256)
    ).copy()
    return {"xpad": xpad, "wstk": wstk, "biasin": biasin}


def kernel(x, W, b):
    if "nc" not in _cache:
        _cache["nc"] = _build_program()
    nc = _cache["nc"]
    in_maps = []
    for c in range(N_CORES):
        x_core = x[c * B_LOC : (c + 1) * B_LOC]
        in_maps.append(_prep_core_inputs(x_core, W, b))
    res = run_bass_kernel_spmd(nc, in_maps, list(range(N_CORES)))
    outs = [res.results[c]["out"] for c in range(N_CORES)]
    full = np.concatenate(outs, axis=0)  # [B, H, W, Ao, Co] fp16
    return np.ascontiguousarray(full.transpose(0, 1, 2, 4, 3)).astype(np.float32)


if __name__ == "__main__":
    x = np.random.randn(16, 32, 32, 8, 32).astype(np.float32)
    W = np.random.randn(5, 5, 32, 256).astype(np.float32) * np.sqrt(2.0 / 800)
    b = np.full((1, 1, 16, 16), 0.1, dtype=np.float32)
    out = kernel(x, W, b)
    print(out.shape, out.dtype)


# revision 13
# speedup vs baseline: 1.0275x; 1.0275x over previous
"""ConvCapsuleLayer Trainium2 kernel (v3).

Strategy:
  - Data-parallel over batch B=16 across 8 cores (B_local=2 per core).
  - Conv (5x5, SAME, Ai=32 -> Co*Ao=256) done on the PE as x-stationary
    matmuls: lhsT = shifted x patches [K=(tap,ai), M=128 pixels],
    rhs = repacked W [K, 256] fp16, accumulated over 7 tap-groups in PSUM.
    The PE runs at a fixed ~277ns per 256-column matmul (hw power cap),
    so matmul COUNT is what matters: the iter-1 "sum over ci" plane is
    computed on-chip with a DVE tree instead of an extra conv plane.
  - Emitted in row chunks (8,8,16 rows for batch 0; 16,16 for batch 1) so
    the first routing chunk starts after ~1/8 of the conv, hiding the
    routing pipeline fill behind the conv.
  - Votes layout [pix, g, ao, ci, co] fp16 (co innermost): big DVE
    multiplies hit 2x mode (stride-1 innermost on all operands) and all
    reductions (over ci for preact, over ao for agreement/norm) are
    contiguous tree-adds instead of strided tensor_reduces.
  - Routing (3 iters) per pixel on-chip: softmax over co (fp16 logits,
    f32 exp), preact = sum_ci route*votes + bias, squash factor per
    (pixel, co), agreement = sum_ao votes*preact with the squash factor
    folded in afterwards. sqrt via exp(0.5*ln(x)); iter-1's 1/16 route
    scale folded into the Exp bias.
  - Output fp16 in (ao, co) order; host transposes to (co, ao) and casts.
"""

import os
import sys

import numpy as np

sys.path.insert(0, "/opt/trn_rl_repo")

from contextlib import ExitStack

import concourse.bacc as bacc
import concourse.bass as bass
import concourse.mybir as mybir
import concourse.tile as tile
from concourse.bass_utils import run_bass_kernel_spmd

F16 = mybir.dt.float16
F32 = mybir.dt.float32
AX = mybir.AxisListType
OP = mybir.AluOpType
AF = mybir.ActivationFunctionType

N_CORES = 8
B_FULL, H, Wd, Ci, Ai = 16, 32, 32, 8, 32
K, Co, Ao = 5, 16, 16
B_LOC = B_FULL // N_CORES  # 2

# (bb, first_row_quad, n_row_quads) chunks; rows = 4*quads
CHUNKS = [(0, 0, 2), (0, 2, 2), (0, 4, 4), (1, 0, 4), (1, 4, 4)]

_cache = {}


def _build_program():
    nc = bacc.Bacc(None, target_bir_lowering=False)
    xpad_d = nc.dram_tensor(
        "xpad", [B_LOC, Ci, 2, 4, Ai, 36, 32], F16, kind="ExternalInput"
    )
    wstk_d = nc.dram_tensor("wstk", [7, 128, 256], F16, kind="ExternalInput")
    bias_d = nc.dram_tensor("biasin", [128, 2, 256], F32, kind="ExternalInput")
    out_d = nc.dram_tensor("out", [B_LOC, H, Wd, Ao, Co], F16, kind="ExternalOutput")

    with tile.TileContext(nc) as tc, ExitStack() as ctx:
        const_p = ctx.enter_context(tc.tile_pool(name="const", bufs=1))
        votes_p = ctx.enter_context(tc.tile_pool(name="votes", bufs=1))
        xrep_p = ctx.enter_context(tc.tile_pool(name="xrep", bufs=3))
        psum_p = ctx.enter_context(
            tc.tile_pool(name="psum", bufs=8, space=bass.MemorySpace.PSUM)
        )
        big_p = ctx.enter_context(tc.tile_pool(name="big", bufs=2))
        tree_p = ctx.enter_context(tc.tile_pool(name="tree", bufs=1))
        mid_p = ctx.enter_context(tc.tile_pool(name="mid", bufs=2))
        tiny_p = ctx.enter_context(tc.tile_pool(name="tiny", bufs=2))
        out_p = ctx.enter_context(tc.tile_pool(name="outs", bufs=2))

        # ---- constants ----
        wstk = const_p.tile([128, 7, 256], F16)
        for s in range(7):
            nc.sync.dma_start(wstk[:, s], wstk_d[s])
        bias_rep = const_p.tile([128, 2, 256], F32)
        nc.sync.dma_start(bias_rep[:], bias_d[:])
        # (ao, co) order; bias16 = 16*b (iter-1 raw preact), bias1 = b
        bias_h = const_p.tile([128, 2, 256], F16)
        nc.scalar.copy(bias_h[:], bias_rep[:])
        bias16 = bias_h[:, 0].rearrange("p (ao co) -> p ao co", ao=16)
        bias1 = bias_h[:, 1].rearrange("p (ao co) -> p ao co", ao=16)
        ln16 = const_p.tile([128, 1], F32)
        nc.gpsimd.memset(ln16[:], float(np.log(1.0 / 16.0)))

        # votes per chunk: [pix, g, ao, ci, co] fp16
        votes_c = [
            votes_p.tile([128, ng, 16, Ci, 16], F16, name=f"votes{i}")
            for i, (_, _, ng) in enumerate(CHUNKS)
        ]

        def conv_chunk(idx):
            bb, g0, ng = CHUNKS[idx]
            r0 = 4 * g0  # first padded row needed (rows r0 .. r0+4*ng+3)
            ncols = (4 * ng + 4) * 32
            for ci in range(Ci):
                xrep = xrep_p.tile([128, ncols], F16, tag="xrepA")
                xrep2 = xrep_p.tile([128, ncols], F16, tag="xrepB")
                src = xpad_d[bb, ci, 0].rearrange("s ai r c -> (s ai) (r c)")
                nc.sync.dma_start(xrep[:], src[:, r0 * 32 : r0 * 32 + ncols])
                src2 = xpad_d[bb, ci, 1].rearrange("s ai r c -> (s ai) (r c)")
                nc.sync.dma_start(xrep2[:], src2[:, r0 * 32 : r0 * 32 + ncols])
                for yq in range(ng):
                    ps = psum_p.tile([128, 256], F32, tag="convps")
                    for dy in range(5):
                        o = (4 * yq + dy) * 32
                        nc.tensor.matmul(
                            ps[:],
                            xrep[:, o : o + 128],
                            wstk[:, dy],
                            start=(dy == 0),
                            stop=False,
                        )
                    o = 4 * yq * 32
                    nc.tensor.matmul(
                        ps[:], xrep2[:, o : o + 128], wstk[:, 5], start=False,
                        stop=False,
                    )
                    o = (4 * yq + 4) * 32
                    nc.tensor.matmul(
                        ps[:],
                        xrep2[0:32, o : o + 128],
                        wstk[0:32, 6],
                        start=False,
                        stop=True,
                    )
                    nc.scalar.copy(votes_c[idx][:, yq, :, ci, :], ps[:])

        def squash_factor(ns, ng, scale):
            """fac[g, co] = scale*sqrt(s2*ns)/(1+s2*ns), ns fp16 [128,ng,16].
            The extra output scale (iter-1's 1/16) is folded into Exp's bias."""
            s2 = scale * scale
            lnv = tiny_p.tile([128, ng, 16], F32, tag="lnv")
            nc.scalar.activation(lnv[:], ns[:], AF.Ln, scale=s2)
            sqr = tiny_p.tile([128, ng, 16], F32, tag="sqr")
            nc.scalar.activation(
                sqr[:], lnv[:], AF.Exp, scale=0.5,
                bias=ln16[:] if scale != 1.0 else 0.0,
            )
            onep = tiny_p.tile([128, ng, 16], F32, tag="onep")
            nc.vector.tensor_scalar(onep[:], ns[:], s2, 1.0, op0=OP.mult, op1=OP.add)
            rec = tiny_p.tile([128, ng, 16], F32, tag="rec")
            nc.vector.reciprocal(rec[:], onep[:])
            fac = tiny_p.tile([128, ng, 16], F32, tag="fac")
            nc.vector.tensor_tensor(fac[:], sqr[:], rec[:], OP.mult)
            return fac

        def ns_tree(sq, ng):
            """ns[g, co] = sum_ao sq[g, ao, co], contiguous tree adds."""
            n1 = tiny_p.tile([128, ng, 8, 16], F16, tag="ns1")
            nc.vector.tensor_tensor(n1[:], sq[:, :, 0:8], sq[:, :, 8:16], OP.add)
            n2 = tiny_p.tile([128, ng, 4, 16], F16, tag="ns2")
            nc.vector.tensor_tensor(n2[:], n1[:, :, 0:4], n1[:, :, 4:8], OP.add)
            n3 = tiny_p.tile([128, ng, 2, 16], F16, tag="ns3")
            nc.vector.tensor_tensor(n3[:], n2[:, :, 0:2], n2[:, :, 2:4], OP.add)
            ns = tiny_p.tile([128, ng, 16], F16, tag="ns")
            nc.vector.tensor_tensor(ns[:], n3[:, :, 0], n3[:, :, 1], OP.add)
            return ns

        def agreement_tree(V, pb, ng):
            """agr0[g, ci, co] fp16 = sum_ao V * pb (pb broadcast over ci)."""
            t2 = big_p.tile([128, ng, 16, Ci, 16], F16, tag="big")
            pbb = pb[:].unsqueeze(3).broadcast_to([128, ng, 16, Ci, 16])
            nc.vector.tensor_tensor(t2[:], V, pbb, OP.mult)
            a1 = tree_p.tile([128, ng, 8, Ci, 16], F16, tag="atr1")
            nc.vector.tensor_tensor(a1[:], t2[:, :, 0:8], t2[:, :, 8:16], OP.add)
            a2 = tree_p.tile([128, ng, 4, Ci, 16], F16, tag="atr2")
            nc.vector.tensor_tensor(a2[:], a1[:, :, 0:4], a1[:, :, 4:8], OP.add)
            a3 = tree_p.tile([128, ng, 2, Ci, 16], F16, tag="atr3")
            nc.vector.tensor_tensor(a3[:], a2[:, :, 0:2], a2[:, :, 2:4], OP.add)
            agr0 = mid_p.tile([128, ng, Ci, 16], F16, tag="agr0")
            nc.vector.tensor_tensor(agr0[:], a3[:, :, 0], a3[:, :, 1], OP.add)
            return agr0

        def ci_tree(t1, ng, bias_ap):
            """pb[g, ao, co] fp16 = sum_ci t1 + bias."""
            p1 = tree_p.tile([128, ng, 16, 4, 16], F16, tag="ptr1")
            nc.vector.tensor_tensor(
                p1[:], t1[:, :, :, 0:4], t1[:, :, :, 4:8], OP.add
            )
            p2 = tree_p.tile([128, ng, 16, 2, 16], F16, tag="ptr2")
            nc.vector.tensor_tensor(
                p2[:], p1[:, :, :, 0:2], p1[:, :, :, 2:4], OP.add
            )
            p3 = tree_p.tile([128, ng, 16, 16], F16, tag="ptr3")
            nc.vector.tensor_tensor(p3[:], p2[:, :, :, 0], p2[:, :, :, 1], OP.add)
            pb = mid_p.tile([128, ng, 16, 16], F16, tag="pb")
            bb_ = bias_ap.unsqueeze(1).broadcast_to([128, ng, 16, 16])
            nc.vector.tensor_tensor(pb[:], p3[:], bb_, OP.add)
            return pb

        def weighted_preact(V, r, ng, bias_ap):
            t1 = big_p.tile([128, ng, 16, Ci, 16], F16, tag="big")
            rb = r[:].unsqueeze(2).broadcast_to([128, ng, 16, Ci, 16])
            nc.vector.tensor_tensor(t1[:], V, rb, OP.mult)
            return ci_tree(t1[:], ng, bias_ap)

        def softmax_route(lg, ng):
            """r[g, ci, co] fp16 = softmax over co of lg fp16 (exp in f32)."""
            e = mid_p.tile([128, ng, Ci, 16], F32, tag="expv")
            nc.scalar.activation(e[:], lg[:], AF.Exp)
            den = tiny_p.tile([128, ng, Ci], F32, tag="den")
            nc.vector.tensor_reduce(den[:], e[:], axis=AX.X, op=OP.add)
            rc = tiny_p.tile([128, ng, Ci], F32, tag="rc")
            nc.vector.reciprocal(rc[:], den[:])
            r = mid_p.tile([128, ng, Ci, 16], F16, tag="route")
            rcb = rc[:].unsqueeze(3).broadcast_to([128, ng, Ci, 16])
            nc.vector.tensor_tensor(r[:], e[:], rcb, OP.mult)
            return r

        def routing_chunk(idx):
            bb, g0, ng = CHUNKS[idx]
            V = votes_c[idx][:]
            inv16 = 1.0 / 16.0

            # ---- iter 1 (uniform route; raw preact = sum_ci V + 16*bias) ----
            pb1 = ci_tree(votes_c[idx][:], ng, bias16)
            sq1 = mid_p.tile([128, ng, 16, 16], F16, tag="sq")
            nc.scalar.activation(sq1[:], pb1[:], AF.Square)
            ns1 = ns_tree(sq1, ng)
            # fac1 includes the 1/16 route scale (Exp bias = ln(1/16))
            fac1 = squash_factor(ns1, ng, inv16)
            agr1 = agreement_tree(V, pb1, ng)
            logits1 = mid_p.tile([128, ng, Ci, 16], F16, tag="logits1")
            f1b = fac1[:].unsqueeze(2).broadcast_to([128, ng, Ci, 16])
            nc.vector.tensor_tensor(logits1[:], agr1[:], f1b, OP.mult)

            # ---- iter 2 ----
            r2 = softmax_route(logits1, ng)
            pb2 = weighted_preact(V, r2, ng, bias1)
            sq2 = mid_p.tile([128, ng, 16, 16], F16, tag="sq")
            nc.scalar.activation(sq2[:], pb2[:], AF.Square)
            ns2 = ns_tree(sq2, ng)
            fac2 = squash_factor(ns2, ng, 1.0)
            agr2 = agreement_tree(V, pb2, ng)
            upd = mid_p.tile([128, ng, Ci, 16], F16, tag="upd")
            f2b = fac2[:].unsqueeze(2).broadcast_to([128, ng, Ci, 16])
            nc.vector.tensor_tensor(upd[:], agr2[:], f2b, OP.mult)
            logits2 = mid_p.tile([128, ng, Ci, 16], F16, tag="logits2")
            nc.vector.tensor_tensor(logits2[:], logits1[:], upd[:], OP.add)

            # ---- iter 3 ----
            r3 = softmax_route(logits2, ng)
            pb3 = weighted_preact(V, r3, ng, bias1)
            sq3 = mid_p.tile([128, ng, 16, 16], F16, tag="sq")
            nc.scalar.activation(sq3[:], pb3[:], AF.Square)
            ns3 = ns_tree(sq3, ng)
            fac3 = squash_factor(ns3, ng, 1.0)
            act3 = out_p.tile([128, ng, 16, 16], F16, tag="actout")
            f3b = fac3[:].unsqueeze(2).broadcast_to([128, ng, 16, 16])
            nc.vector.tensor_tensor(act3[:], pb3[:], f3b, OP.mult)

            rows0 = 4 * g0
            dst = out_d[bb, rows0 : rows0 + 4 * ng].rearrange(
                "(gg yy) x ao co -> (yy x) gg ao co", yy=4
            )
            # ACT queue, not sync: the sync queue carries the next chunk's
            # xrep loads and must not stall behind routing completion.
            nc.scalar.dma_start(dst, act3[:])

        for i in range(len(CHUNKS)):
            conv_chunk(i)
            routing_chunk(i)

    nc.compile()
    return nc


def _prep_core_inputs(x_core, W, b):
    f16 = np.float16
    planes = np.transpose(x_core, (0, 3, 4, 1, 2)).astype(f16)  # [B,Ci,Ai,H,W]
    # xpad[b, 0, s, ai, ci, r, c] = plane[r-2, c+s-2]   (s = dx shift 0..3)
    # xpad[b, 1, g, ai, ci, r, c] = plane[r+g-2, c+2]   (g = dy shift 0..3, dx=4)
    xpad = np.zeros((B_LOC, Ci, 2, 4, Ai, 36, 32), dtype=f16)
    for s in range(4):
        c_lo = max(0, 2 - s)
        c_hi = min(32, 34 - s)
        xpad[:, :, 0, s, :, 2:34, c_lo:c_hi] = planes[
            :, :, :, :, c_lo + s - 2 : c_hi + s - 2
        ]
    for g in range(4):
        r_lo = max(0, 2 - g)
        r_hi = min(36, 34 - g)
        xpad[:, :, 1, g, :, r_lo:r_hi, 0:30] = planes[
            :, :, :, r_lo + g - 2 : r_hi + g - 2, 2:32
        ]
    # W stacks in (ao, co) output order:
    # slot dy (0..4): [(dx g, ai), 256]; slot 5: [(dy g, ai), 256] at dx=4;
    # slot 6: [ai, 256] for tap (4, 4).
    Wr = W.reshape(K, K, Ai, Co, Ao).transpose(0, 1, 2, 4, 3)  # [dy,dx,ai,ao,co]
    wstk = np.zeros((7, 128, 256), dtype=f16)
    for dy in range(5):
        wstk[dy] = (
            Wr[dy, 0:4].reshape(4 * Ai, Ao * Co).astype(f16)
        )  # [(dx,ai), (ao,co)]
    wstk[5] = Wr[0:4, 4].reshape(4 * Ai, Ao * Co).astype(f16)  # [(dy,ai), ...]
    wstk[6, :32] = Wr[4, 4].reshape(Ai, Ao * Co).astype(f16)
    bias_aoco = b[0, 0].T.reshape(256).astype(np.float32)  # (ao, co) order
    biasin = np.broadcast_to(
        np.stack([16.0 * bias_aoco, bias_aoco])[None], (128, 2, 256)
    ).copy()
    return {"xpad": xpad, "wstk": wstk, "biasin": biasin}


def kernel(x, W, b):
    if "nc" not in _cache:
        _cache["nc"] = _build_program()
    nc = _cache["nc"]
    in_maps = []
    for c in range(N_CORES):
        x_core = x[c * B_LOC : (c + 1) * B_LOC]
        in_maps.append(_prep_core_inputs(x_core, W, b))
    res = run_bass_kernel_spmd(nc, in_maps, list(range(N_CORES)))
    outs = [res.results[c]["out"] for c in range(N_CORES)]
    full = np.concatenate(outs, axis=0)  # [B, H, W, Ao, Co] fp16
    return np.ascontiguousarray(full.transpose(0, 1, 2, 4, 3)).astype(np.float32)


if __name__ == "__main__":
    x = np.random.randn(16, 32, 32, 8, 32).astype(np.float32)
    W = np.random.randn(5, 5, 32, 256).astype(np.float32) * np.sqrt(2.0 / 800)
    b = np.full((1, 1, 16, 16), 0.1, dtype=np.float32)
    out = kernel(x, W, b)
    print(out.shape, out.dtype)
